# revision 52
# baseline (speedup 1.0000x reference)
"""Trainium2 Bass kernel for linear attention (ELU+1 feature map) block:
Q/K/V projections + linear attention + out-projection + residual + LayerNorm,
distributed over 8 NeuronCores.

Sharding: 8-way row split of the (batch*seq) dimension. Cores 2b and 2b+1
process the two 2048-row halves of batch b; the per-(batch,head) global
reductions KtQ [H,64,64] and q_sum [D] are pair-AllReduced on device.

v2: fp8e4 DoubleRow matmuls for all four projections (4x PE throughput),
q_sum folded into KtQ via an appended ones column, K kept resident in SBUF
(no HBM spill), xq kept resident for the residual, bf16 output (host cast),
residual added via identity-matmul into the attention psum, LN via bn_stats.
"""
import os
import sys

for _p in ("/opt/trn_rl_repo", "/root/.axon_site/_ro/trn_rl_repo"):
    if os.path.isdir(_p) and _p not in sys.path:
        sys.path.insert(0, _p)

import numpy as np
import ml_dtypes

B, N, D, H = 4, 4096, 1024, 16
DEPTH = D // H  # 64
NCORES = 8
R = (B * N) // NCORES  # 2048 rows per core
NSUB = R // 128  # 16 token subtiles per core
NBLK = R // 512  # 4 token blocks per core
EPS_LN = 1e-6
HP = 65  # per-head KtQ stationary width: 64 K cols + 1 ones col

_CACHE = {}


def _build(zb_qk, zb_v, zb_o, g_one, b_zero, single=False):
    import concourse.bacc as bacc
    import concourse.tile as tile
    from concourse import mybir
    from concourse.masks import make_identity
    from contextlib import ExitStack

    F32 = mybir.dt.float32
    F32R = mybir.dt.float32r
    BF16 = mybir.dt.bfloat16
    FP8 = mybir.dt.float8e4
    ALU = mybir.AluOpType
    AF = mybir.ActivationFunctionType
    PM = mybir.MatmulPerfMode
    AX = mybir.AxisListType

    nc = bacc.Bacc("TRN2", debug=False, num_devices=1 if single else NCORES)

    xq = nc.dram_tensor("xq", [R, D], BF16, kind="ExternalInput").ap()
    xk = nc.dram_tensor("xk", [R, D], BF16, kind="ExternalInput").ap()
    xv = nc.dram_tensor("xv", [R, D], BF16, kind="ExternalInput").ap()
    wq = nc.dram_tensor("wq", [D, D], FP8, kind="ExternalInput").ap()
    wk = nc.dram_tensor("wk", [D, D], FP8, kind="ExternalInput").ap()
    wv = nc.dram_tensor("wv", [D, D], BF16, kind="ExternalInput").ap()
    wo = nc.dram_tensor("wo", [D, D], FP8, kind="ExternalInput").ap()
    bq = nc.dram_tensor("bq", [1, D], F32, kind="ExternalInput").ap()
    bk = nc.dram_tensor("bk", [1, D], F32, kind="ExternalInput").ap()
    bv = nc.dram_tensor("bv", [1, D], F32, kind="ExternalInput").ap()
    bo = nc.dram_tensor("bo", [1, D], F32, kind="ExternalInput").ap()
    gamma = nc.dram_tensor("gamma", [1, D], F32, kind="ExternalInput").ap()
    beta = nc.dram_tensor("beta", [1, D], F32, kind="ExternalInput").ap()
    out = nc.dram_tensor("out", [R, D], BF16, kind="ExternalOutput").ap()

    with tile.TileContext(nc) as tc, ExitStack() as ctx:
        const_p = ctx.enter_context(tc.tile_pool(name="const", bufs=1))
        resid = ctx.enter_context(tc.tile_pool(name="resid", bufs=1))
        wpool = ctx.enter_context(tc.tile_pool(name="wpool", bufs=1))
        wstage = ctx.enter_context(tc.tile_pool(name="wstage", bufs=2))
        dp = ctx.enter_context(tc.tile_pool(name="dram", bufs=1, space="DRAM"))

        # ---- constants ----
        ident_f = wstage.tile([128, 1024], F32, tag="wst", name="ident_f")
        make_identity(nc, ident_f[:, 0:128])
        identr = const_p.tile([128, 128], F32R, tag="identr")
        nc.scalar.copy(identr[:], ident_f[:, 0:128])
        identb = const_p.tile([128, 128], BF16, tag="identb")
        nc.scalar.copy(identb[:], ident_f[:, 0:128])

        eps_ln = const_p.tile([128, 1], F32, tag="eps_ln")
        nc.gpsimd.memset(eps_ln[:], EPS_LN)

        # S selection matrix: S[h, x] = 1 iff h == x // 64 (bf16)
        s_f = wstage.tile([16, D], F32, tag="wst", name="s_f", padded_shape=[128, D])
        nc.gpsimd.memset(s_f[:], 0.0)
        s_f3 = s_f[:].rearrange("h (j l) -> h j l", l=64)
        nc.gpsimd.affine_select(
            out=s_f3,
            in_=s_f3,
            compare_op=ALU.not_equal,
            fill=1.0,
            base=0,
            pattern=[[-1, 16], [0, 64]],
            channel_multiplier=1,
        )
        s_b = const_p.tile([16, D], BF16, tag="s_b")
        nc.scalar.copy(s_b[:], s_f[:])

        def bcast_row(name, src_ap):
            row = const_p.tile([1, D], F32, tag=name + "_row")
            nc.sync.dma_start(row[:], src_ap)
            bc = const_p.tile([128, D], F32, tag=name + "_bc")
            nc.gpsimd.partition_broadcast(bc[:], row[:])
            return bc

        bq_bc = None if zb_qk else bcast_row("bq", bq)
        bk_bc = None if zb_qk else bcast_row("bk", bk)
        bo_bc = None if zb_o else bcast_row("bo", bo)
        gamma_bc = None if g_one else bcast_row("gamma", gamma)
        beta_bc = None if b_zero else bcast_row("beta", beta)
        bv_pp = None
        if not zb_v:
            # per-partition bias for feature-major V: bv_pp[p, c] = bv[c*128+p]
            bv_pp = const_p.tile([128, 8], F32, tag="bv_pp")
            for c in range(8):
                nc.sync.dma_start(
                    bv_pp[:, c : c + 1], bv[0:1, c * 128 : (c + 1) * 128]
                )

        # fp8 weight pair tiles: w8[k][p, i, f] = W[k*256 + i*128 + p, f]
        def make_w8(name, dt=FP8, pool=None):
            pool = pool or wpool
            return [
                pool.tile([128, 2, D], dt, tag=f"{name}{k}", name=f"{name}{k}")
                for k in range(4)
            ]

        def load_w8_pair(w_ap, w8, k):
            # w8[k][p, i, f] = W[k*256 + i*128 + p, f]
            src_ap = w_ap[k * 256 : (k + 1) * 256, :].rearrange(
                "(i p) f -> p i f", p=128
            )
            nc.sync.dma_start(w8[k][:], src_ap)

        def load_w8(w_ap, w8, name=None):
            for k in range(4):
                load_w8_pair(w_ap, w8, k)

        # collective scratch: rows 0-63 KtQ [d, h*64+e]; row 64 q_sum
        cc_in = dp.tile([HP, D], F32, tag="cc_in")
        cc_out = dp.tile([HP, D], F32, tag="cc_out")

        # =========================== PHASE A ===========================
        # residents
        xqres = [
            resid.tile([128, D], BF16, tag=f"xqres{s}", name=f"xqres{s}")
            for s in range(NSUB)
        ]
        k8 = [
            resid.tile([128, H * HP], BF16, tag=f"k8_{s}", name=f"k8_{s}")
            for s in range(NSUB)
        ]

        from contextlib import ExitStack as _ES

        with _ES() as actx:
            xknat = actx.enter_context(tc.tile_pool(name="xknat", bufs=5))
            xt8 = actx.enter_context(tc.tile_pool(name="xt8", bufs=6))
            q8p = actx.enter_context(tc.tile_pool(name="q8p", bufs=5))
            elu_t = actx.enter_context(tc.tile_pool(name="elu_t", bufs=3))
            psTr = actx.enter_context(tc.tile_pool(name="psTr", bufs=2, space="PSUM"))
            psQ = actx.enter_context(tc.tile_pool(name="psQ", bufs=2, space="PSUM"))
            psK = actx.enter_context(tc.tile_pool(name="psK", bufs=2, space="PSUM"))
            psKtq = actx.enter_context(tc.tile_pool(name="psKtq", bufs=1, space="PSUM"))

            wqk_pool = actx.enter_context(tc.tile_pool(name="wqk", bufs=1))
            wq8 = make_w8("wq8", pool=wqk_pool)
            wk8 = make_w8("wk8", pool=wqk_pool)
            wv8 = make_w8("wv8", BF16)
            wo8 = make_w8("wo8")

            # ones columns of k8 (col 64 of each head block)
            for s in range(NSUB):
                k3 = k8[s][:].rearrange("p (h e) -> p h e", e=HP)
                nc.gpsimd.memset(k3[:, :, 64:65], 1.0)

            ktq_ps = psKtq.tile([HP, D], F32, tag="ktq_ps")
            for half in range(2):
                nc.vector.memset(ktq_ps[:, half * 512 : (half + 1) * 512], 0.0)

            def transpose_x(x_nat, name):
                """x_nat [128 tok, 1024] f32r -> two fp8 pair tiles
                [128, 2, 256]: g covers d-slabs 4g..4g+3 as pairs (2g, 2g+1).
                tile[:, i, j*128:(j+1)*128] = slab (4g + 2j + i)^T."""
                groups = []
                for g in range(2):
                    ps_t = psTr.tile([128, 512], BF16, tag="trA")
                    for j in range(4):
                        c = g * 4 + j
                        nc.tensor.transpose(
                            ps_t[:, j * 128 : (j + 1) * 128],
                            x_nat[:, c * 128 : (c + 1) * 128],
                            identb[:],
                        )
                    t8 = xt8.tile([128, 2, 256], FP8, tag=f"xt8{name[:3]}", name=f"{name}{g}")
                    # psum order [4g, 4g+1, 4g+2, 4g+3] -> out [i, j, e]
                    src = ps_t[:].rearrange("p (j i e) -> p i j e", i=2, e=128)
                    dst = t8[:].rearrange("p i (j e) -> p i j e", e=128)
                    if name.startswith("xkT"):
                        nc.vector.tensor_copy(dst, src)
                    else:
                        nc.scalar.copy(dst, src)
                    groups.append(t8)
                return groups

            def project_elu(xt_groups, w8, ps_pool, tag, dst3, bias_bc, name):
                """Half-width DoubleRow projection + fused ELU+1 per half:
                dst3 [128, 16, 64] view; half h covers head-blocks 8h..8h+7."""
                for half in range(2):
                    sl = slice(half * 512, (half + 1) * 512)
                    ph = ps_pool.tile(
                        [128, 512], F32, tag=tag, name=f"{name}_h{half}"
                    )
                    for k in range(4):
                        lhsT = xt_groups[k // 2][
                            :, :, (k % 2) * 128 : (k % 2 + 1) * 128
                        ]
                        nc.tensor.matmul(
                            ph[:],
                            lhsT,
                            w8[k][:, :, sl],
                            start=(k == 0),
                            stop=(k == 3),
                            perf_mode=PM.DoubleRow,
                        )
                    src0 = ph[:]
                    if bias_bc is not None:
                        xb = elu_t.tile(
                            [128, 512], F32, tag="xb", name=f"xb_{name}{half}"
                        )
                        nc.vector.tensor_tensor(xb[:], ph[:], bias_bc[:, sl], ALU.add)
                        src0 = xb[:]
                    e = elu_t.tile([128, 512], BF16, tag="e", name=f"e_{name}{half}")
                    nc.scalar.activation(e[:], src0, AF.Exp)
                    em1 = elu_t.tile(
                        [128, 512], BF16, tag="em1", name=f"em1_{name}{half}"
                    )
                    nc.gpsimd.tensor_scalar_min(em1[:], e[:], 1.0)
                    src3 = src0.rearrange("p (h e) -> p h e", e=64)
                    em3 = em1[:].rearrange("p (h e) -> p h e", e=64)
                    nc.vector.scalar_tensor_tensor(
                        dst3[:, half * 8 : (half + 1) * 8, :], src3, 1.0, em3,
                        op0=ALU.add, op1=ALU.max,
                    )

            # staggered software pipeline: K-side lags Q-side by KLAG subtiles
            # so the wk (and Q-side wq) weight loads never stall the pipe.
            KLAG = 0
            xk_nats = {}
            xTq = {}
            xTk = {}
            q8s = {}

            def stage_loads(s):
                rows = slice(s * 128, (s + 1) * 128)
                nc.sync.dma_start(xqres[s][:], xq[rows, :])
                xk_nats[s] = xknat.tile([128, D], BF16, tag="xk_nat", name=f"xk{s}")
                nc.sync.dma_start(xk_nats[s][:], xk[rows, :])

            load_w8(wq, wq8)
            stage_loads(0)
            load_w8(wk, wk8)
            stage_loads(1)
            stage_loads(2)
            stage_loads(3)
            stage_loads(4)

            def stage_transpose(s):
                xTq[s] = transpose_x(xqres[s][:], f"xqT{s}_")
                xTk[s] = transpose_x(xk_nats[s][:], f"xkT{s}_")

            def stage_project_q(s):
                q8 = q8p.tile([128, D], BF16, tag="q8", name=f"q8_{s}")
                project_elu(
                    xTq.pop(s), wq8, psQ, "ps_q",
                    q8[:].rearrange("p (h e) -> p h e", e=64), bq_bc, f"q{s}",
                )
                q8s[s] = q8

            def stage_project_k(s):
                k3 = k8[s][:].rearrange("p (h e) -> p h e", e=HP)
                project_elu(xTk.pop(s), wk8, psK, "ps_k", k3[:, :, 0:64], bk_bc, f"k{s}")

            def stage_ktq(s):
                q8 = q8s.pop(s)
                q3 = q8[:].rearrange("p (h e) -> p h e", e=64)
                k3 = k8[s][:].rearrange("p (h e) -> p h e", e=HP)
                for h in range(H):
                    nc.tensor.matmul(
                        ktq_ps[:, h * 64 : (h + 1) * 64],
                        k3[:, h, :],
                        q3[:, h, :],
                        start=False,
                        stop=(s == NSUB - 1),
                        skip_group_check=True,
                    )

            for it in range(NSUB + KLAG + 2):
                if it < NSUB:
                    if it >= 5:
                        stage_loads(it)
                    stage_transpose(it)
                if 1 <= it <= NSUB:
                    stage_project_q(it - 1)
                if KLAG + 1 <= it <= NSUB + KLAG:
                    stage_project_k(it - 1 - KLAG)
                if it >= KLAG + 2:
                    stage_ktq(it - 2 - KLAG)
            # wv loads fill the phase-A tail / transition DMA gap; wo follows
            load_w8(wv, wv8, "wv")

            ktq_sb = wstage.tile(
                [HP, D], F32, tag="wst", name="ktq_sb", padded_shape=[128, D]
            )
            for half in range(2):
                sl = slice(half * 512, (half + 1) * 512)
                nc.scalar.copy(ktq_sb[:, sl], ktq_ps[:, sl])
            nc.sync.dma_start(cc_in[:], ktq_sb[:])

        if single:
            nc.sync.dma_start(cc_out[:], cc_in[:])
        else:
            nc.gpsimd.collective_compute(
                "AllReduce",
                ALU.add,
                replica_groups=[[0, 1], [2, 3], [4, 5], [6, 7]],
                ins=[cc_in.opt()],
                outs=[cc_out.opt()],
            )

        # =========================== PHASE B ===========================
        with _ES() as bctx:
            glob = bctx.enter_context(tc.tile_pool(name="glob", bufs=1))
            xvnat = bctx.enter_context(tc.tile_pool(name="xvnat", bufs=2))
            xvT_p = bctx.enter_context(tc.tile_pool(name="xvT", bufs=2))
            vsb_p = bctx.enter_context(tc.tile_pool(name="vsb", bufs=2))
            zt_p = bctx.enter_context(tc.tile_pool(name="zt", bufs=2))
            div_p = bctx.enter_context(tc.tile_pool(name="divsb", bufs=2))
            oh_p = bctx.enter_context(tc.tile_pool(name="oh8", bufs=2))
            ln_p = bctx.enter_context(tc.tile_pool(name="ln", bufs=2))
            y_p = bctx.enter_context(tc.tile_pool(name="y", bufs=2))
            small = bctx.enter_context(tc.tile_pool(name="small", bufs=2))
            psTrB = bctx.enter_context(tc.tile_pool(name="psTrB", bufs=1, space="PSUM"))
            psV = bctx.enter_context(tc.tile_pool(name="psV", bufs=1, space="PSUM"))
            psNum = bctx.enter_context(tc.tile_pool(name="psNum", bufs=2, space="PSUM"))
            psDiv = bctx.enter_context(tc.tile_pool(name="psDiv", bufs=1, space="PSUM"))
            psAttn = bctx.enter_context(tc.tile_pool(name="psAttn", bufs=2, space="PSUM"))

            wo_loaded = False

            # block-diagonal KtQ (bf16): pair c -> [128,128] block
            ktq_f = wstage.tile([128, D], F32, tag="wst", name="ktq_f")
            nc.gpsimd.memset(ktq_f[:], 0.0)
            bd3 = ktq_f[:].rearrange("p (c e) -> p c e", e=128)
            cc3 = cc_out[0:64, :].rearrange("p (c t e) -> p c t e", t=2, e=64)
            nc.sync.dma_start(bd3[0:64, :, 0:64], cc3[:, :, 0, :])
            nc.sync.dma_start(bd3[64:128, :, 64:128], cc3[:, :, 1, :])
            ktq_b = glob.tile([128, D], BF16, tag="ktq_b")
            nc.scalar.copy(ktq_b[:], ktq_f[:])

            qs_row = wstage.tile(
                [1, D], F32, tag="wst", name="qs_row", padded_shape=[128, D]
            )
            nc.sync.dma_start(qs_row[:], cc_out[64:65, :])
            qs_rowb = wstage.tile(
                [1, D], BF16, tag="wst", name="qs_rowb", padded_shape=[128, D]
            )
            nc.vector.tensor_copy(qs_rowb[:], qs_row[:])
            qsum_bc = glob.tile([128, D], BF16, tag="qsum_bc")
            nc.gpsimd.partition_broadcast(qsum_bc[:], qs_rowb[:])

            # ---- Z = K . q_sum per token/head; DVE part per block ----
            iz_fm = glob.tile([16, R], BF16, tag="iz_fm")
            iz_bs = {}

            def z_dve(blk):
                for t in range(4):
                    s = blk * 4 + t
                    k3 = k8[s][:].rearrange("p (h e) -> p h e", e=HP)
                    prod = zt_p.tile([128, D], BF16, tag="prod", name=f"prod{s}")
                    p3 = prod[:].rearrange("p (h e) -> p h e", e=64)
                    nc.vector.tensor_tensor(p3, k3[:, :, 0:64],
                        qsum_bc[:].rearrange("p (h e) -> p h e", e=64), ALU.mult)
                    z_t = zt_p.tile([128, 16], F32, tag="z_t", name=f"z_t{s}")
                    nc.vector.tensor_reduce(z_t[:], p3, AX.X, ALU.add)
                    iz_t = zt_p.tile([128, 16], F32, tag="iz_t", name=f"iz_t{s}")
                    nc.vector.reciprocal(iz_t[:], z_t[:])
                    iz_b = zt_p.tile([128, 16], BF16, tag="iz_b", name=f"iz_b{s}")
                    nc.vector.tensor_copy(iz_b[:], iz_t[:])
                    iz_bs[s] = iz_b

            def z_pe(blk):
                ps_zt = psTrB.tile([16, 512], BF16, tag="ps_zt", name=f"ps_zt{blk}")
                for t in range(4):
                    s = blk * 4 + t
                    nc.tensor.transpose(
                        ps_zt[:, t * 128 : (t + 1) * 128], iz_bs.pop(s)[:], identb[:]
                    )
                nc.scalar.copy(iz_fm[:, blk * 512 : (blk + 1) * 512], ps_zt[:])

            z_dve(0)
            z_dve(1)

            for blk in range(NBLK):
                # ---- xv loads ----
                xv_nats = []
                for t in range(4):
                    rows = slice(blk * 512 + t * 128, blk * 512 + (t + 1) * 128)
                    xv_nat = xvnat.tile(
                        [128, D], BF16, tag=f"xv_nat{t}", name=f"xv_{blk}_{t}"
                    )
                    nc.sync.dma_start(xv_nat[:], xv[rows, :])
                    xv_nats.append(xv_nat)
                if not wo_loaded:
                    load_w8(wo, wo8, "wo")
                    wo_loaded = True

                # ---- V transposes -> fp8 pairs [128, 2, 512] per d-pair ----
                xvT = []
                for g in range(4):
                    ps_t = psTrB.tile([128, 512], BF16, tag="trB", name=f"trB{blk}_{g}")
                    for j in range(4):
                        nc.tensor.transpose(
                            ps_t[:, j * 128 : (j + 1) * 128],
                            xv_nats[j][:, (g * 2) * 128 : (g * 2 + 1) * 128],
                            identb[:],
                        )
                    ps_t2 = psTrB.tile(
                        [128, 512], BF16, tag="trB", name=f"trB2{blk}_{g}"
                    )
                    for j in range(4):
                        nc.tensor.transpose(
                            ps_t2[:, j * 128 : (j + 1) * 128],
                            xv_nats[j][:, (g * 2 + 1) * 128 : (g * 2 + 2) * 128],
                            identb[:],
                        )
                    t8 = xvT_p.tile([128, 2, 512], BF16, tag=f"xvT{g}", name=f"xvT{blk}_{g}")
                    nc.scalar.copy(t8[:, 0, :], ps_t[:])
                    nc.scalar.copy(t8[:, 1, :], ps_t2[:])
                    xvT.append(t8)

                if blk + 2 < NBLK:
                    z_dve(blk + 2)
                z_pe(blk)

                # ---- V projection (feature-major, DoubleRow) ----
                v_sb = []
                for c in range(8):
                    ps_v = psV.tile([128, 512], F32, tag="ps_v")
                    for k in range(4):
                        for i in range(2):
                            nc.tensor.matmul(
                                ps_v[:],
                                wv8[k][:, i, c * 128 : (c + 1) * 128],
                                xvT[k][:, i, :],
                                start=(k == 0 and i == 0),
                                stop=(k == 3 and i == 1),
                            )
                    vt = vsb_p.tile([128, 512], BF16, tag=f"v_sb{c}", name=f"v{blk}_{c}")
                    if zb_v:
                        nc.scalar.copy(vt[:], ps_v[:])
                    else:
                        nc.scalar.activation(
                            vt[:], ps_v[:], AF.Identity, bias=bv_pp[:, c : c + 1]
                        )
                    v_sb.append(vt)

                # ---- numerator + divisor + OH (fp8 pairs for out-proj) ----
                oh8 = [
                    oh_p.tile([128, 2, 512], FP8, tag=f"oh8{p}", name=f"oh{blk}_{p}")
                    for p in range(4)
                ]
                for c in range(8):
                    ps_d = psDiv.tile([128, 512], F32, tag="ps_d")
                    nc.tensor.matmul(
                        ps_d[:],
                        s_b[:, c * 128 : (c + 1) * 128],
                        iz_fm[:, blk * 512 : (blk + 1) * 512],
                        start=True,
                        stop=True,
                    )
                    div_sb = div_p.tile([128, 512], BF16, tag="div_sb")
                    nc.scalar.copy(div_sb[:], ps_d[:])
                    ps_n = psNum.tile([128, 512], F32, tag="ps_n")
                    nc.tensor.matmul(
                        ps_n[:],
                        ktq_b[:, c * 128 : (c + 1) * 128],
                        v_sb[c][:],
                        start=True,
                        stop=True,
                    )
                    nc.vector.scalar_tensor_tensor(
                        oh8[c // 2][:, c % 2, :],
                        ps_n[:],
                        1.0,
                        div_sb[:],
                        op0=ALU.mult,
                        op1=ALU.mult,
                    )

                # ---- out-projection + residual + LayerNorm ----
                for t in range(4):
                    s = blk * 4 + t
                    rows = slice(s * 128, (s + 1) * 128)
                    res = xqres[s][:]
                    if bo_bc is not None:
                        qb = ln_p.tile([128, D], F32, tag="qb", name=f"qb{blk}_{t}")
                        nc.vector.tensor_tensor(qb[:], res, bo_bc[:], ALU.add)
                        res = qb[:]
                    x_sb = ln_p.tile([128, D], BF16, tag="x_sb", name=f"x{blk}_{t}")
                    bst = small.tile([128, 2, 6], F32, tag="bst")
                    for half in range(2):
                        sl = slice(half * 512, (half + 1) * 512)
                        ps_a = psAttn.tile(
                            [128, 512], F32, tag="ps_a", name=f"ps_a{blk}_{t}_{half}"
                        )
                        for p in range(4):
                            nc.tensor.matmul(
                                ps_a[:],
                                oh8[p][:, :, t * 128 : (t + 1) * 128],
                                wo8[p][:, :, sl],
                                start=(p == 0),
                                stop=(p == 3),
                                perf_mode=PM.DoubleRow,
                            )
                        nc.vector.scalar_tensor_tensor(
                            x_sb[:, sl], ps_a[:], 1.0, res[:, sl],
                            op0=ALU.mult, op1=ALU.add,
                        )
                        nc.vector.bn_stats(bst[:, half, :], x_sb[:, sl])
                    mv = small.tile([128, 2], F32, tag="mv")
                    nc.vector.bn_aggr(mv[:], bst[:])
                    std = small.tile([128, 1], F32, tag="std")
                    nc.scalar.activation(std[:], mv[:, 1:2], AF.Sqrt, bias=eps_ln[:])
                    rstd = small.tile([128, 1], F32, tag="rstd")
                    nc.vector.reciprocal(rstd[:], std[:])
                    nmu = small.tile([128, 1], F32, tag="nmu")
                    nc.vector.tensor_scalar(
                        nmu[:], mv[:, 0:1], rstd[:], -1.0,
                        op0=ALU.mult, op1=ALU.mult,
                    )
                    y = y_p.tile([128, D], BF16, tag="y")
                    nc.scalar.activation(
                        y[:], x_sb[:], AF.Identity, bias=nmu[:], scale=rstd[:]
                    )
                    if not g_one:
                        nc.vector.tensor_tensor(y[:], y[:], gamma_bc[:], ALU.mult)
                    if not b_zero:
                        nc.vector.tensor_tensor(y[:], y[:], beta_bc[:], ALU.add)
                    nc.gpsimd.dma_start(out[rows, :], y[:])

    nc.compile()
    return nc


def _get_nc(flags):
    if flags not in _CACHE:
        _CACHE[flags] = _build(*flags)
    return _CACHE[flags]


def _prep(inputs):
    q = np.ascontiguousarray(
        np.asarray(inputs["query"], dtype=np.float32).astype(ml_dtypes.bfloat16)
    )
    k = np.ascontiguousarray(
        np.asarray(inputs["key"], dtype=np.float32).astype(ml_dtypes.bfloat16)
    )
    v = np.ascontiguousarray(
        np.asarray(inputs["value"], dtype=np.float32).astype(ml_dtypes.bfloat16)
    )
    Wq = np.ascontiguousarray(
        np.asarray(inputs["Wq"], dtype=np.float32).astype(ml_dtypes.float8_e4m3)
    )
    Wk = np.ascontiguousarray(
        np.asarray(inputs["Wk"], dtype=np.float32).astype(ml_dtypes.float8_e4m3)
    )
    Wv = np.ascontiguousarray(
        np.asarray(inputs["Wv"], dtype=np.float32).astype(ml_dtypes.bfloat16)
    )
    Wo = np.ascontiguousarray(
        np.asarray(inputs["Wo"], dtype=np.float32).astype(ml_dtypes.float8_e4m3)
    )
    bqv = np.ascontiguousarray(np.asarray(inputs["bq"], dtype=np.float32).reshape(1, D))
    bkv = np.ascontiguousarray(np.asarray(inputs["bk"], dtype=np.float32).reshape(1, D))
    bvv = np.ascontiguousarray(np.asarray(inputs["bv"], dtype=np.float32).reshape(1, D))
    bov = np.ascontiguousarray(np.asarray(inputs["bo"], dtype=np.float32).reshape(1, D))
    gv = np.ascontiguousarray(np.asarray(inputs["gamma"], dtype=np.float32).reshape(1, D))
    btv = np.ascontiguousarray(np.asarray(inputs["beta"], dtype=np.float32).reshape(1, D))

    flags = (
        bool(not bqv.any() and not bkv.any()),
        bool(not bvv.any()),
        bool(not bov.any()),
        bool(np.all(gv == 1.0)),
        bool(not btv.any()),
    )
    qf = q.reshape(NCORES, R, D)
    kf = k.reshape(NCORES, R, D)
    vf = v.reshape(NCORES, R, D)
    in_maps = []
    for c in range(NCORES):
        in_maps.append(
            {
                "xq": qf[c], "xk": kf[c], "xv": vf[c],
                "wq": Wq, "wk": Wk, "wv": Wv, "wo": Wo,
                "bq": bqv, "bk": bkv, "bv": bvv, "bo": bov,
                "gamma": gv, "beta": btv,
            }
        )
    return flags, in_maps


def kernel(**inputs):
    from concourse.bass_utils import run_bass_kernel_spmd

    flags, in_maps = _prep(inputs)
    nc = _get_nc(flags)
    res = run_bass_kernel_spmd(nc, in_maps, core_ids=list(range(NCORES)))
    outs = np.stack(
        [np.asarray(res.results[c]["out"]).astype(np.float32) for c in range(NCORES)],
        axis=0,
    )
    return outs.reshape(B, N, D)
